# revision 9
# baseline (speedup 1.0000x reference)
"""Trainium2 Bass kernel for nn_FCNetwork3D (batch-1 dense CNN+MLP).

Network: x[1,2264] -> 6x Conv3d(1,1,3,SAME)+ReLU on the 6x6x6 tail ->
concat -> normalize -> Linear(2264,4096)+tanh -> Linear(4096,4096)+tanh
-> Linear(4096,32) -> scale/shift.

Sharding (8 cores): tensor-parallel on the two wide Linears.
  L0 column-parallel: core k computes h0 block k [512] (weights pre-
    transposed+normalization-folded on host), tanh locally.
  AllGather h0 (2KB/core) on-device.
  L1 column-parallel: core k computes h1 block k [512], tanh locally.
  L2 row-parallel over h1 blocks: core k computes a partial [1,32]
    (out_scale folded into weights, bias/out_shift split /8 across
    cores); host unshard = sum of the 8 partials.
The tiny conv stack runs replicated on every core as 6 matvecs with
host-built [216,216] conv matrices (pure weight placement, im2col-style).

Matmuls use fp32r (fp32 rounded to 11-bit mantissa by the PE datapath)
for the two big streams: 1 cycle/row at N>=256 vs 4 for plain fp32.
The conv matvecs (N=1, illegal for fp32r) stay plain fp32.
"""

import numpy as np

import concourse.bass as bass
import concourse.mybir as mybir
import concourse.tile as tile
from concourse import bacc
from concourse import bass_utils

F32 = mybir.dt.float32
F32R = mybir.dt.float32r
AF = mybir.ActivationFunctionType

NCORES = 8
OBS, ACTD, H, VOX = 2264, 32, 4096, 216
XH = OBS - VOX            # 2048 (x head)
S = H // NCORES           # 512 (per-core block of the hidden dim)
KC0 = XH // 128           # 16 x-head K-chunks
KC1 = H // 128            # 32 h0 K-chunks


def build_nc(reps: int = 1, fake_gather: bool = False):
    """Build the per-core Bass program (identical on all 8 cores; data
    differs via per-core inputs). reps>1 unrolls the whole body for
    steady-state throughput measurement. fake_gather replaces the
    AllGather with a DRAM round-trip + an hrest input (single-core
    TimelineSim oracle)."""
    nc = bacc.Bacc("TRN2", target_bir_lowering=False, debug=False,
                   num_devices=1 if fake_gather else NCORES)

    xh_d = nc.dram_tensor("xh", [XH], F32, kind="ExternalInput")
    v0_d = nc.dram_tensor("v0", [VOX], F32, kind="ExternalInput")
    ct_d = nc.dram_tensor("ct", [6, VOX, VOX], F32, kind="ExternalInput")
    cb_d = nc.dram_tensor("cb", [6], F32, kind="ExternalInput")
    one_d = nc.dram_tensor("onec", [1], F32, kind="ExternalInput")
    a0_d = nc.dram_tensor("a0", [OBS + 1, S], F32, kind="ExternalInput")
    a1_d = nc.dram_tensor("a1", [H + 1, S], F32, kind="ExternalInput")
    a2_d = nc.dram_tensor("a2", [S + 1, ACTD], F32, kind="ExternalInput")
    hrest_d = (nc.dram_tensor("hrest", [H], F32, kind="ExternalInput")
               if fake_gather else None)
    y_d = nc.dram_tensor("y", [1, ACTD], F32, kind="ExternalOutput")

    with tile.TileContext(nc) as tc:
        with (
            tc.tile_pool(name="wp", bufs=6) as wp,
            tc.tile_pool(name="cp", bufs=24) as cp,
            tc.tile_pool(name="sp", bufs=4) as sp,
            tc.tile_pool(name="ps", bufs=2, space="PSUM") as ps,
            tc.tile_pool(name="psa", bufs=1, space="PSUM") as psa,
            tc.tile_pool(name="dr", bufs=2, space="DRAM") as dr,
        ):
            for _ in range(reps):
                # ---- constants ----
                one_t = sp.tile([1, 1], F32R)
                nc.sync.dma_start(out=one_t[:],
                                  in_=one_d.ap().unsqueeze(-1).bitcast(F32R))
                cbb = sp.tile([128, 6], F32)
                nc.sync.dma_start(out=cbb[:],
                                  in_=cb_d.ap().unsqueeze(0).to_broadcast((128, 6)))

                # x head as 16 column-chunks [128, 16]
                xt = sp.tile([128, KC0], F32R)
                nc.sync.dma_start(
                    out=xt[:],
                    in_=xh_d.ap().rearrange("(c p) -> p c", p=128).bitcast(F32R))

                # ---- conv stack: 6 serial matvecs in plain fp32 ----
                vc0 = sp.tile([128, 1], F32)
                vc1 = sp.tile([88, 1], F32)
                nc.sync.dma_start(out=vc0[:], in_=v0_d[0:128].unsqueeze(-1))
                nc.sync.dma_start(out=vc1[:], in_=v0_d[128:VOX].unsqueeze(-1))
                for i in range(6):
                    w00 = cp.tile([128, 128], F32)
                    w10 = cp.tile([88, 128], F32)
                    w01 = cp.tile([128, 88], F32)
                    w11 = cp.tile([88, 88], F32)
                    nc.sync.dma_start(out=w00[:], in_=ct_d[i, 0:128, 0:128])
                    nc.sync.dma_start(out=w10[:], in_=ct_d[i, 128:VOX, 0:128])
                    nc.sync.dma_start(out=w01[:], in_=ct_d[i, 0:128, 128:VOX])
                    nc.sync.dma_start(out=w11[:], in_=ct_d[i, 128:VOX, 128:VOX])
                    pm0 = ps.tile([128, 1], F32)
                    pm1 = ps.tile([88, 1], F32)
                    nc.tensor.matmul(pm0[:], w00[:], vc0[:], start=True, stop=False)
                    nc.tensor.matmul(pm0[:], w10[:], vc1[:], start=False, stop=True)
                    nc.tensor.matmul(pm1[:], w01[:], vc0[:], start=True, stop=False)
                    nc.tensor.matmul(pm1[:], w11[:], vc1[:], start=False, stop=True)
                    nv0 = sp.tile([128, 1], F32)
                    nv1 = sp.tile([88, 1], F32)
                    nc.scalar.activation(nv0[:], pm0[:], AF.Relu,
                                         bias=cbb[:, i:i + 1])
                    nc.scalar.activation(nv1[:], pm1[:], AF.Relu,
                                         bias=cbb[0:88, i:i + 1])
                    vc0, vc1 = nv0, nv1
                # conv output to fp32r for the L0 matmuls
                cv0 = sp.tile([128, 1], F32R)
                cv1 = sp.tile([88, 1], F32R)
                nc.scalar.copy(cv0[:], vc0[:])
                nc.scalar.copy(cv1[:], vc1[:])

                # ---- L0: h0_blk = tanh(xn @ A0 + b0_blk)  [1, 512] ----
                # weight stream in 1MB grouped DMAs (4 K-chunks per tile)
                G = 4
                ph0 = psa.tile([1, S], F32)
                for g in range(KC0 // G):
                    wt = wp.tile([128, G * S], F32R)
                    nc.sync.dma_start(
                        out=wt[:].rearrange("p (j e) -> p j e", j=G),
                        in_=a0_d[g * G * 128:(g + 1) * G * 128, :]
                        .rearrange("(j p) e -> p j e", p=128).bitcast(F32R))
                    for j in range(G):
                        c = g * G + j
                        nc.tensor.matmul(ph0[:], xt[:, c:c + 1],
                                         wt[:, j * S:(j + 1) * S],
                                         start=(c == 0), stop=False)
                wts = wp.tile([128, S], F32R)
                nc.sync.dma_start(out=wts[:],
                                  in_=a0_d[XH:XH + 128, :].bitcast(F32R))
                nc.tensor.matmul(ph0[:], cv0[:], wts[:], start=False, stop=False)
                wts = wp.tile([88, S], F32R)
                nc.sync.dma_start(out=wts[:],
                                  in_=a0_d[XH + 128:OBS, :].bitcast(F32R))
                nc.tensor.matmul(ph0[:], cv1[:], wts[:], start=False, stop=False)
                a0b = sp.tile([1, S], F32R)
                nc.sync.dma_start(out=a0b[:], in_=a0_d[OBS:OBS + 1, :].bitcast(F32R))
                nc.tensor.matmul(ph0[:], one_t[:], a0b[:], start=False, stop=True)
                h0s = sp.tile([1, S], F32)
                nc.scalar.activation(h0s[:], ph0[:], AF.Tanh)

                # ---- AllGather h0 blocks -> full h0 [4096] ----
                h0g = sp.tile([128, KC1], F32R)
                if fake_gather:
                    ccin = dr.tile([S], F32)
                    nc.gpsimd.dma_start(out=ccin[:], in_=h0s[:])
                    nc.sync.dma_start(
                        out=h0g[:, 0:S // 128],
                        in_=ccin[:].rearrange("(c p) -> p c", p=128).bitcast(F32R))
                    nc.sync.dma_start(
                        out=h0g[:, S // 128:KC1],
                        in_=hrest_d[S:H].rearrange("(c p) -> p c", p=128).bitcast(F32R))
                else:
                    ccin = dr.tile([S], F32)
                    ccout = dr.tile([H], F32)
                    nc.gpsimd.dma_start(out=ccin[:], in_=h0s[:])
                    nc.gpsimd.collective_compute(
                        "AllGather", mybir.AluOpType.bypass,
                        replica_groups=[list(range(NCORES))],
                        ins=[ccin[:].opt()], outs=[ccout[:].opt()])
                    nc.sync.dma_start(
                        out=h0g[:],
                        in_=ccout[:].rearrange("(c p) -> p c", p=128).bitcast(F32R))

                # ---- L1: h1_blk = tanh(h0 @ A1 + b1_blk)  [1, 512] ----
                ph1 = psa.tile([1, S], F32)
                for g in range(KC1 // G):
                    wt = wp.tile([128, G * S], F32R)
                    nc.sync.dma_start(
                        out=wt[:].rearrange("p (j e) -> p j e", j=G),
                        in_=a1_d[g * G * 128:(g + 1) * G * 128, :]
                        .rearrange("(j p) e -> p j e", p=128).bitcast(F32R))
                    for j in range(G):
                        c = g * G + j
                        nc.tensor.matmul(ph1[:], h0g[:, c:c + 1],
                                         wt[:, j * S:(j + 1) * S],
                                         start=(c == 0), stop=False)
                a1b = sp.tile([1, S], F32R)
                nc.sync.dma_start(out=a1b[:], in_=a1_d[H:H + 1, :].bitcast(F32R))
                nc.tensor.matmul(ph1[:], one_t[:], a1b[:], start=False, stop=True)
                h1s = sp.tile([1, S], F32)
                nc.scalar.activation(h1s[:], ph1[:], AF.Tanh)

                # ---- L2 partial: y_k = h1_blk @ A2_blk + bias'/8  [1, 32] ----
                # h1 row -> column chunks via a DRAM round-trip
                hscr = dr.tile([S], F32)
                nc.gpsimd.dma_start(out=hscr[:], in_=h1s[:])
                h1g = sp.tile([128, S // 128], F32R)
                nc.sync.dma_start(
                    out=h1g[:],
                    in_=hscr[:].rearrange("(c p) -> p c", p=128).bitcast(F32R))
                py = psa.tile([1, ACTD], F32)
                for c in range(S // 128):
                    at = sp.tile([128, ACTD], F32R)
                    nc.sync.dma_start(
                        out=at[:], in_=a2_d[c * 128:(c + 1) * 128, :].bitcast(F32R))
                    nc.tensor.matmul(py[:], h1g[:, c:c + 1], at[:],
                                     start=(c == 0), stop=False)
                a2b = sp.tile([1, ACTD], F32R)
                nc.sync.dma_start(out=a2b[:], in_=a2_d[S:S + 1, :].bitcast(F32R))
                nc.tensor.matmul(py[:], one_t[:], a2b[:], start=False, stop=True)
                ys = sp.tile([1, ACTD], F32)
                nc.scalar.copy(ys[:], py[:])
                nc.sync.dma_start(out=y_d[:, :], in_=ys[:])

    nc.compile()
    return nc


def _conv_matrix(w: np.ndarray) -> np.ndarray:
    """[216,216] dense matrix of a 3x3x3 SAME cross-correlation on a
    6x6x6 grid: C[o, i] such that y.flat = C @ v.flat."""
    w = np.asarray(w, dtype=np.float32).reshape(3, 3, 3)
    C = np.zeros((VOX, VOX), dtype=np.float32)
    idx = np.arange(6)
    for dz in (-1, 0, 1):
        for dy in (-1, 0, 1):
            for dx in (-1, 0, 1):
                zo, zi = idx[max(0, -dz):6 - max(0, dz)], idx[max(0, dz):6 - max(0, -dz)]
                yo, yi = idx[max(0, -dy):6 - max(0, dy)], idx[max(0, dy):6 - max(0, -dy)]
                xo, xi = idx[max(0, -dx):6 - max(0, dx)], idx[max(0, dx):6 - max(0, -dx)]
                o = (zo[:, None, None] * 36 + yo[None, :, None] * 6 + xo[None, None, :]).ravel()
                i = (zi[:, None, None] * 36 + yi[None, :, None] * 6 + xi[None, None, :]).ravel()
                C[o, i] = w[dz + 1, dy + 1, dx + 1]
    return C


def make_in_maps(inputs: dict) -> list[dict]:
    """Host-side layout prep + sharding: fold normalization into A0,
    out_scale/shift into A2, pre-transpose weights, build conv matrices."""
    f = np.float32
    x = np.asarray(inputs["x"], f)
    W0, b0 = np.asarray(inputs["W0"], f), np.asarray(inputs["b0"], f)
    W1, b1 = np.asarray(inputs["W1"], f), np.asarray(inputs["b1"], f)
    W2, b2 = np.asarray(inputs["W2"], f), np.asarray(inputs["b2"], f)
    in_shift = np.asarray(inputs["in_shift"], f)
    in_scale = np.asarray(inputs["in_scale"], f)
    out_shift = np.asarray(inputs["out_shift"], f)
    out_scale = np.asarray(inputs["out_scale"], f)

    sc = (1.0 / (in_scale.astype(np.float64) + 1e-8)).astype(f)       # [2264]
    A0 = (W0 * sc[None, :]).T.astype(f)                               # [2264, 4096]
    bias0 = (b0 - (in_shift * sc) @ W0.T).astype(f)                   # [4096]
    A1 = W1.T.astype(f)                                               # [4096, 4096]
    A2 = (W2.T * out_scale[None, :]).astype(f)                        # [4096, 32]
    bias2 = ((b2 * out_scale + out_shift) / NCORES).astype(f)         # [32]

    ct = np.stack([_conv_matrix(inputs[f"cw{i}"]).T for i in range(6)])  # [6,216,216]
    cb = np.array([np.asarray(inputs[f"cb{i}"], f).ravel()[0]
                   for i in range(6)], f)

    xh = np.ascontiguousarray(x.ravel()[:XH])
    v0 = np.ascontiguousarray(x.ravel()[XH:])
    onec = np.ones([1], f)

    in_maps = []
    for k in range(NCORES):
        blk = slice(k * S, (k + 1) * S)
        a0 = np.concatenate([A0[:, blk], bias0[blk][None, :]], axis=0)
        a1 = np.concatenate([A1[:, blk], b1[blk][None, :]], axis=0)
        a2 = np.concatenate([A2[blk, :], bias2[None, :]], axis=0)
        in_maps.append(dict(
            xh=xh, v0=v0, ct=ct, cb=cb, onec=onec,
            a0=np.ascontiguousarray(a0),
            a1=np.ascontiguousarray(a1),
            a2=np.ascontiguousarray(a2),
        ))
    return in_maps


_NC_CACHE: dict = {}


def kernel(**inputs) -> np.ndarray:
    if "nc" not in _NC_CACHE:
        _NC_CACHE["nc"] = build_nc(reps=1)
    nc = _NC_CACHE["nc"]
    in_maps = make_in_maps(inputs)
    res = bass_utils.run_bass_kernel_spmd(nc, in_maps,
                                          core_ids=list(range(NCORES)))
    y = np.sum([res.results[k]["y"] for k in range(NCORES)], axis=0)
    return y.astype(np.float32)


# revision 10
# speedup vs baseline: 1.2009x; 1.2009x over previous
"""Trainium2 Bass kernel for nn_FCNetwork3D (batch-1 dense CNN+MLP).

Network: x[1,2264] -> 6x Conv3d(1,1,3,SAME)+ReLU on the 6x6x6 tail ->
concat -> normalize -> Linear(2264,4096)+tanh -> Linear(4096,4096)+tanh
-> Linear(4096,32) -> scale/shift.

Sharding (8 cores): tensor-parallel on the two wide Linears.
  L0 column-parallel: core k computes h0 block k [512] (weights pre-
    transposed+normalization-folded on host), tanh locally.
  AllGather h0 (2KB/core) on-device.
  L1 column-parallel: core k computes h1 block k [512], tanh locally.
  L2 row-parallel over h1 blocks: core k computes a partial [1,32]
    (out_scale folded into weights, bias/out_shift split /8 across
    cores); host unshard = sum of the 8 partials.
The tiny conv stack runs replicated on every core as 6 matvecs with
host-built [216,216] conv matrices (pure weight placement, im2col-style).

Matmuls use fp32r (fp32 rounded to 11-bit mantissa by the PE datapath)
for the two big streams: 1 cycle/row at N>=256 vs 4 for plain fp32.
The conv matvecs (N=1, illegal for fp32r) stay plain fp32.
"""

import numpy as np

import concourse.bass as bass
import concourse.mybir as mybir
import concourse.tile as tile
from concourse import bacc
from concourse import bass_utils

F32 = mybir.dt.float32
F32R = mybir.dt.float32r
AF = mybir.ActivationFunctionType

NCORES = 8
OBS, ACTD, H, VOX = 2264, 32, 4096, 216
XH = OBS - VOX            # 2048 (x head)
S = H // NCORES           # 512 (per-core block of the hidden dim)
KC0 = XH // 128           # 16 x-head K-chunks
KC1 = H // 128            # 32 h0 K-chunks


def build_nc(reps: int = 1, fake_gather: bool = False):
    """Build the per-core Bass program (identical on all 8 cores; data
    differs via per-core inputs). reps>1 unrolls the whole body for
    steady-state throughput measurement. fake_gather replaces the
    AllGather with a DRAM round-trip + an hrest input (single-core
    TimelineSim oracle)."""
    nc = bacc.Bacc("TRN2", target_bir_lowering=False, debug=False,
                   num_devices=1 if fake_gather else NCORES)

    xh_d = nc.dram_tensor("xh", [XH], F32, kind="ExternalInput")
    v0_d = nc.dram_tensor("v0", [VOX], F32, kind="ExternalInput")
    ct_d = nc.dram_tensor("ct", [6, VOX, VOX], F32, kind="ExternalInput")
    cb_d = nc.dram_tensor("cb", [6], F32, kind="ExternalInput")
    one_d = nc.dram_tensor("onec", [1], F32, kind="ExternalInput")
    a0_d = nc.dram_tensor("a0", [OBS + 1, S], F32, kind="ExternalInput")
    a1_d = nc.dram_tensor("a1", [H + 1, S], F32, kind="ExternalInput")
    a2_d = nc.dram_tensor("a2", [S + 1, ACTD], F32, kind="ExternalInput")
    hrest_d = (nc.dram_tensor("hrest", [H], F32, kind="ExternalInput")
               if fake_gather else None)
    y_d = nc.dram_tensor("y", [1, ACTD], F32, kind="ExternalOutput")

    with tile.TileContext(nc) as tc:
        with (
            tc.tile_pool(name="wp", bufs=6) as wp,
            tc.tile_pool(name="cp", bufs=24) as cp,
            tc.tile_pool(name="sp", bufs=4) as sp,
            tc.tile_pool(name="ps", bufs=2, space="PSUM") as ps,
            tc.tile_pool(name="psa", bufs=1, space="PSUM") as psa,
            tc.tile_pool(name="dr", bufs=2, space="DRAM") as dr,
        ):
            for _ in range(reps):
                # ---- constants ----
                one_t = sp.tile([1, 1], F32R)
                nc.sync.dma_start(out=one_t[:],
                                  in_=one_d.ap().unsqueeze(-1).bitcast(F32R))
                cbb = sp.tile([128, 6], F32)
                nc.sync.dma_start(out=cbb[:],
                                  in_=cb_d.ap().unsqueeze(0).to_broadcast((128, 6)))

                # x head as 16 column-chunks [128, 16]
                xt = sp.tile([128, KC0], F32R)
                nc.sync.dma_start(
                    out=xt[:],
                    in_=xh_d.ap().rearrange("(c p) -> p c", p=128).bitcast(F32R))

                # ---- conv stack: 6 serial matvecs in plain fp32 ----
                vc0 = sp.tile([128, 1], F32)
                vc1 = sp.tile([88, 1], F32)
                nc.sync.dma_start(out=vc0[:], in_=v0_d[0:128].unsqueeze(-1))
                nc.sync.dma_start(out=vc1[:], in_=v0_d[128:VOX].unsqueeze(-1))
                for i in range(6):
                    w00 = cp.tile([128, 128], F32)
                    w10 = cp.tile([88, 128], F32)
                    w01 = cp.tile([128, 88], F32)
                    w11 = cp.tile([88, 88], F32)
                    nc.sync.dma_start(out=w00[:], in_=ct_d[i, 0:128, 0:128])
                    nc.sync.dma_start(out=w10[:], in_=ct_d[i, 128:VOX, 0:128])
                    nc.sync.dma_start(out=w01[:], in_=ct_d[i, 0:128, 128:VOX])
                    nc.sync.dma_start(out=w11[:], in_=ct_d[i, 128:VOX, 128:VOX])
                    pm0 = ps.tile([128, 1], F32)
                    pm1 = ps.tile([88, 1], F32)
                    nc.tensor.matmul(pm0[:], w00[:], vc0[:], start=True, stop=False)
                    nc.tensor.matmul(pm0[:], w10[:], vc1[:], start=False, stop=True)
                    nc.tensor.matmul(pm1[:], w01[:], vc0[:], start=True, stop=False)
                    nc.tensor.matmul(pm1[:], w11[:], vc1[:], start=False, stop=True)
                    nv0 = sp.tile([128, 1], F32)
                    nv1 = sp.tile([88, 1], F32)
                    nc.scalar.activation(nv0[:], pm0[:], AF.Relu,
                                         bias=cbb[:, i:i + 1])
                    nc.scalar.activation(nv1[:], pm1[:], AF.Relu,
                                         bias=cbb[0:88, i:i + 1])
                    vc0, vc1 = nv0, nv1
                # conv output to fp32r for the L0 matmuls
                cv0 = sp.tile([128, 1], F32R)
                cv1 = sp.tile([88, 1], F32R)
                nc.scalar.copy(cv0[:], vc0[:])
                nc.scalar.copy(cv1[:], vc1[:])

                # ---- L0: h0_blk = tanh(xn @ A0 + b0_blk)  [1, 512] ----
                # weight stream in 1MB grouped DMAs (4 K-chunks per tile)
                G = 4
                ph0 = psa.tile([1, S], F32)
                for g in range(KC0 // G):
                    wt = wp.tile([128, G * S], F32R)
                    weng = nc.sync if g % 2 == 0 else nc.scalar
                    weng.dma_start(
                        out=wt[:].rearrange("p (j e) -> p j e", j=G),
                        in_=a0_d[g * G * 128:(g + 1) * G * 128, :]
                        .rearrange("(j p) e -> p j e", p=128).bitcast(F32R))
                    for j in range(G):
                        c = g * G + j
                        nc.tensor.matmul(ph0[:], xt[:, c:c + 1],
                                         wt[:, j * S:(j + 1) * S],
                                         start=(c == 0), stop=False)
                wts = wp.tile([128, S], F32R)
                nc.scalar.dma_start(out=wts[:],
                                    in_=a0_d[XH:XH + 128, :].bitcast(F32R))
                nc.tensor.matmul(ph0[:], cv0[:], wts[:], start=False, stop=False)
                wts = wp.tile([88, S], F32R)
                nc.sync.dma_start(out=wts[:],
                                  in_=a0_d[XH + 128:OBS, :].bitcast(F32R))
                nc.tensor.matmul(ph0[:], cv1[:], wts[:], start=False, stop=False)
                a0b = sp.tile([1, S], F32R)
                nc.sync.dma_start(out=a0b[:], in_=a0_d[OBS:OBS + 1, :].bitcast(F32R))
                nc.tensor.matmul(ph0[:], one_t[:], a0b[:], start=False, stop=True)
                h0s = sp.tile([1, S], F32)
                nc.scalar.activation(h0s[:], ph0[:], AF.Tanh)

                # ---- AllGather h0 blocks -> full h0 [4096] ----
                h0g = sp.tile([128, KC1], F32R)
                if fake_gather:
                    ccin = dr.tile([S], F32)
                    nc.gpsimd.dma_start(out=ccin[:], in_=h0s[:])
                    nc.sync.dma_start(
                        out=h0g[:, 0:S // 128],
                        in_=ccin[:].rearrange("(c p) -> p c", p=128).bitcast(F32R))
                    nc.sync.dma_start(
                        out=h0g[:, S // 128:KC1],
                        in_=hrest_d[S:H].rearrange("(c p) -> p c", p=128).bitcast(F32R))
                else:
                    ccin = dr.tile([S], F32)
                    ccout = dr.tile([H], F32)
                    nc.gpsimd.dma_start(out=ccin[:], in_=h0s[:])
                    nc.gpsimd.collective_compute(
                        "AllGather", mybir.AluOpType.bypass,
                        replica_groups=[list(range(NCORES))],
                        ins=[ccin[:].opt()], outs=[ccout[:].opt()])
                    nc.sync.dma_start(
                        out=h0g[:],
                        in_=ccout[:].rearrange("(c p) -> p c", p=128).bitcast(F32R))

                # ---- L1: h1_blk = tanh(h0 @ A1 + b1_blk)  [1, 512] ----
                ph1 = psa.tile([1, S], F32)
                for g in range(KC1 // G):
                    wt = wp.tile([128, G * S], F32R)
                    weng = nc.sync if g % 2 == 0 else nc.scalar
                    weng.dma_start(
                        out=wt[:].rearrange("p (j e) -> p j e", j=G),
                        in_=a1_d[g * G * 128:(g + 1) * G * 128, :]
                        .rearrange("(j p) e -> p j e", p=128).bitcast(F32R))
                    for j in range(G):
                        c = g * G + j
                        nc.tensor.matmul(ph1[:], h0g[:, c:c + 1],
                                         wt[:, j * S:(j + 1) * S],
                                         start=(c == 0), stop=False)
                a1b = sp.tile([1, S], F32R)
                nc.sync.dma_start(out=a1b[:], in_=a1_d[H:H + 1, :].bitcast(F32R))
                nc.tensor.matmul(ph1[:], one_t[:], a1b[:], start=False, stop=True)
                h1s = sp.tile([1, S], F32)
                nc.scalar.activation(h1s[:], ph1[:], AF.Tanh)

                # ---- L2 partial: y_k = h1_blk @ A2_blk + bias'/8  [1, 32] ----
                # h1 row -> column chunks via a DRAM round-trip
                hscr = dr.tile([S], F32)
                nc.gpsimd.dma_start(out=hscr[:], in_=h1s[:])
                h1g = sp.tile([128, S // 128], F32R)
                nc.sync.dma_start(
                    out=h1g[:],
                    in_=hscr[:].rearrange("(c p) -> p c", p=128).bitcast(F32R))
                py = psa.tile([1, ACTD], F32)
                for c in range(S // 128):
                    at = sp.tile([128, ACTD], F32R)
                    nc.sync.dma_start(
                        out=at[:], in_=a2_d[c * 128:(c + 1) * 128, :].bitcast(F32R))
                    nc.tensor.matmul(py[:], h1g[:, c:c + 1], at[:],
                                     start=(c == 0), stop=False)
                a2b = sp.tile([1, ACTD], F32R)
                nc.sync.dma_start(out=a2b[:], in_=a2_d[S:S + 1, :].bitcast(F32R))
                nc.tensor.matmul(py[:], one_t[:], a2b[:], start=False, stop=True)
                ys = sp.tile([1, ACTD], F32)
                nc.scalar.copy(ys[:], py[:])
                nc.sync.dma_start(out=y_d[:, :], in_=ys[:])

    nc.compile()
    return nc


def _conv_matrix(w: np.ndarray) -> np.ndarray:
    """[216,216] dense matrix of a 3x3x3 SAME cross-correlation on a
    6x6x6 grid: C[o, i] such that y.flat = C @ v.flat."""
    w = np.asarray(w, dtype=np.float32).reshape(3, 3, 3)
    C = np.zeros((VOX, VOX), dtype=np.float32)
    idx = np.arange(6)
    for dz in (-1, 0, 1):
        for dy in (-1, 0, 1):
            for dx in (-1, 0, 1):
                zo, zi = idx[max(0, -dz):6 - max(0, dz)], idx[max(0, dz):6 - max(0, -dz)]
                yo, yi = idx[max(0, -dy):6 - max(0, dy)], idx[max(0, dy):6 - max(0, -dy)]
                xo, xi = idx[max(0, -dx):6 - max(0, dx)], idx[max(0, dx):6 - max(0, -dx)]
                o = (zo[:, None, None] * 36 + yo[None, :, None] * 6 + xo[None, None, :]).ravel()
                i = (zi[:, None, None] * 36 + yi[None, :, None] * 6 + xi[None, None, :]).ravel()
                C[o, i] = w[dz + 1, dy + 1, dx + 1]
    return C


def make_in_maps(inputs: dict) -> list[dict]:
    """Host-side layout prep + sharding: fold normalization into A0,
    out_scale/shift into A2, pre-transpose weights, build conv matrices."""
    f = np.float32
    x = np.asarray(inputs["x"], f)
    W0, b0 = np.asarray(inputs["W0"], f), np.asarray(inputs["b0"], f)
    W1, b1 = np.asarray(inputs["W1"], f), np.asarray(inputs["b1"], f)
    W2, b2 = np.asarray(inputs["W2"], f), np.asarray(inputs["b2"], f)
    in_shift = np.asarray(inputs["in_shift"], f)
    in_scale = np.asarray(inputs["in_scale"], f)
    out_shift = np.asarray(inputs["out_shift"], f)
    out_scale = np.asarray(inputs["out_scale"], f)

    sc = (1.0 / (in_scale.astype(np.float64) + 1e-8)).astype(f)       # [2264]
    A0 = (W0 * sc[None, :]).T.astype(f)                               # [2264, 4096]
    bias0 = (b0 - (in_shift * sc) @ W0.T).astype(f)                   # [4096]
    A1 = W1.T.astype(f)                                               # [4096, 4096]
    A2 = (W2.T * out_scale[None, :]).astype(f)                        # [4096, 32]
    bias2 = ((b2 * out_scale + out_shift) / NCORES).astype(f)         # [32]

    ct = np.stack([_conv_matrix(inputs[f"cw{i}"]).T for i in range(6)])  # [6,216,216]
    cb = np.array([np.asarray(inputs[f"cb{i}"], f).ravel()[0]
                   for i in range(6)], f)

    xh = np.ascontiguousarray(x.ravel()[:XH])
    v0 = np.ascontiguousarray(x.ravel()[XH:])
    onec = np.ones([1], f)

    in_maps = []
    for k in range(NCORES):
        blk = slice(k * S, (k + 1) * S)
        a0 = np.concatenate([A0[:, blk], bias0[blk][None, :]], axis=0)
        a1 = np.concatenate([A1[:, blk], b1[blk][None, :]], axis=0)
        a2 = np.concatenate([A2[blk, :], bias2[None, :]], axis=0)
        in_maps.append(dict(
            xh=xh, v0=v0, ct=ct, cb=cb, onec=onec,
            a0=np.ascontiguousarray(a0),
            a1=np.ascontiguousarray(a1),
            a2=np.ascontiguousarray(a2),
        ))
    return in_maps


_NC_CACHE: dict = {}


def kernel(**inputs) -> np.ndarray:
    if "nc" not in _NC_CACHE:
        _NC_CACHE["nc"] = build_nc(reps=1)
    nc = _NC_CACHE["nc"]
    in_maps = make_in_maps(inputs)
    res = bass_utils.run_bass_kernel_spmd(nc, in_maps,
                                          core_ids=list(range(NCORES)))
    y = np.sum([res.results[k]["y"] for k in range(NCORES)], axis=0)
    return y.astype(np.float32)
